# revision 82
# baseline (speedup 1.0000x reference)
"""Trainium2 Bass kernel for sparse (top-k) attention.

Reference computation (per row i of N=8192):
    Q = X @ Wq.T + bq ; K = X @ Wk.T + bk ; V = X @ Wv.T + bv
    S = (Q @ K.T) / (temp * sqrt(D)) ; S[i,i] = -1e9
    keep each row's top-k (k=64) scores, mask rest to -1e9, softmax, @ V.

Strategy (8 NeuronCores, SPMD, no collectives):
  - Rows of X sharded across cores (1024 each); K/V computed replicated
    on every core from the full X.
  - Per core the key axis (j) is rotated by c*1024 so the self-diagonal
    lands at compile-time-known columns (j == local row index).
  - Selection-critical matmuls (Q/K projections, Q@K.T) run as 3-term
    bf16 split products (hi*hi + hi*lo + lo*hi): ~1e-6 absolute score
    accuracy at 3 bf16 passes. The 1/(temp*sqrt(D)) scale is folded into
    Wq on the host.
  - Exact per-row k-th-largest threshold: per-row mean/std estimates
    (from q norms + global K moments) give a conservative candidate
    threshold tau0; candidates are stream-compacted (prefix scan +
    gpsimd local_scatter of fp32 values as u16 halves) into a 512-wide
    buffer; 16 rounds of count bisection (ACT Sign+accum counts, DVE
    updates) isolate the exact k-th value.
  - Weights w = exp(s - t) * [s >= t] (bf16), DMA-xbar transposed in
    128x128 blocks, then PE matmul against V augmented with a ones
    column -> numerator and denominator in one PSUM accumulation chain.
  - Stage B is emitted as a 2-phase software pipeline: P1(t) =
    stats/scores/compaction/scatter, P23(t-1) = bisect/weights/AV, so
    Pool scatters, DVE compaction, ACT counts and PE matmuls of
    adjacent row-blocks overlap. The score slab pool has 3 slots (1.5
    row-blocks) to decouple the phases.
"""

import sys

sys.path.insert(0, "/opt/trn_rl_repo/concourse")
sys.path.insert(0, "/opt/trn_rl_repo")

import math
from contextlib import ExitStack
from statistics import NormalDist

import ml_dtypes
import numpy as np

import concourse.bass as bass
import concourse.tile as tile
from concourse import bacc, mybir
from concourse.bass_utils import run_bass_kernel_spmd

F32 = mybir.dt.float32
BF16 = mybir.dt.bfloat16
I16 = mybir.dt.int16
I32 = mybir.dt.int32
F16 = mybir.dt.float16
U16 = mybir.dt.uint16
OP = mybir.AluOpType
ACTF = mybir.ActivationFunctionType
AX = mybir.AxisListType

NEG = -1e9
P = 128

ABLATE_LEVELS = (
    "stats", "scores", "compact_dve", "scatter", "bisect", "weights",
    "transpose", "full")


def _bf16_split(x):
    hi = np.asarray(x, np.float32).astype(ml_dtypes.bfloat16)
    lo = (x - hi.astype(np.float32)).astype(ml_dtypes.bfloat16)
    return np.ascontiguousarray(hi), np.ascontiguousarray(lo)


class Program:
    def __init__(self, n, d, k, ncores, iters=14, reps=1, ablate=None,
                 skip=()):
        assert n % (ncores * P) == 0 and d % P == 0
        self.n, self.d, self.k, self.ncores = n, d, k, ncores
        self.ablate = ABLATE_LEVELS.index(ablate or "full")
        self.skip = frozenset(skip)
        self.reps = reps
        self.R = n // ncores            # rows per core
        self.NIB = self.R // P          # i-blocks per core
        self.DC = d // P                # contraction chunks
        self.JT = 512                   # score matmul tile width
        self.NJT = n // self.JT
        self.JB = n // P                # 128-wide j blocks
        self.HW = n // 2                # slab half width
        self.QW = n // 4                # compaction quarter width
        self.CW = 128                   # compact slots per quarter (f32)
        self.W = 4 * self.CW            # total compact width
        self.CV = 96                    # counted slots per quarter
        self.VW = d + 2                 # V width incl. ones + pad col
        self.iters = iters
        tgt = min(2.25 * k, 0.45 * self.W)
        nd = NormalDist()
        self.z0 = nd.inv_cdf(1.0 - tgt / n)
        self.zhi = self.z0 + 1.2
        self.build()

    def build(self):
        n, d, k = self.n, self.d, self.k
        DC, NIB, JT, NJT, JB = self.DC, self.NIB, self.JT, self.NJT, self.JB
        HW, QW, CW, W, CV = self.HW, self.QW, self.CW, self.W, self.CV
        R, VW = self.R, self.VW
        ITERS = self.iters
        nc = self.nc = bacc.Bacc(
            "TRN2", target_bir_lowering=False, debug=False,
            num_devices=self.ncores)

        def din(name, shape, dt=BF16):
            return nc.dram_tensor(name, shape, dt, kind="ExternalInput").ap()

        xt_h = din("xt_h", (d, n))
        xt_l = din("xt_l", (d, n))
        wq_h = din("wq_h", (d, d))
        wq_l = din("wq_l", (d, d))
        wk_h = din("wk_h", (d, d))
        wk_l = din("wk_l", (d, d))
        wv_h = din("wv_h", (d, d))
        out_d = nc.dram_tensor("out", (R, d), F32, kind="ExternalOutput").ap()

        with tile.TileContext(nc) as tc, ExitStack() as ctx:
            # ---------------- persistent tensors ----------------
            pers = ctx.enter_context(tc.tile_pool(name="pers", bufs=1))
            kt_h = pers.tile([P, DC * n], BF16, tag="kt_h")
            kt_l = pers.tile([P, DC * n], BF16, tag="kt_l")
            v_sb = pers.tile([P, JB * VW], BF16, tag="v_sb")
            qt_h = pers.tile([P, DC * R], BF16, tag="qt_h")
            qt_l = pers.tile([P, DC * R], BF16, tag="qt_l")
            kbarb = pers.tile([P, DC], BF16, tag="kbarb")
            c_sb = pers.tile([P, DC * d], BF16, tag="c_sb")  # C = K^T K chunks
            conerow = pers.tile([P, ITERS], F32, tag="conerow")
            for it in range(ITERS):
                nc.vector.memset(conerow[:, it:it + 1], 0.5 ** (it + 1))
            onef = pers.tile([P, 1], F32, tag="onef")
            nc.vector.memset(onef[:], 1.0)
            onebf = pers.tile([P, 1], BF16, tag="onebf")
            nc.vector.memset(onebf[:], 1.0)

            def kch(t, c):
                return t[:, c * n:(c + 1) * n]

            def qch(t, c):
                return t[:, c * R:(c + 1) * R]

            # ---------------- stage A ----------------
            with tc.tile_pool(name="xt", bufs=1) as xtp, \
                 tc.tile_pool(name="wp", bufs=1) as wp, \
                 tc.tile_pool(name="pa", bufs=3, space="PSUM") as pa, \
                 tc.tile_pool(name="pc", bufs=1, space="PSUM") as pc:
                xth = xtp.tile([P, DC * n], BF16, tag="xth")
                xtl = xtp.tile([P, DC * n], BF16, tag="xtl")
                for c in range(DC):
                    nc.sync.dma_start(kch(xth, c), xt_h[c * P:(c + 1) * P, :])
                    nc.sync.dma_start(kch(xtl, c), xt_l[c * P:(c + 1) * P, :])
                ws = {}
                for nm, dr in [("wq_h", wq_h), ("wq_l", wq_l),
                               ("wk_h", wk_h), ("wk_l", wk_l),
                               ("wv_h", wv_h)]:
                    t = wp.tile([P, DC * d], BF16, tag=nm)
                    for c in range(DC):
                        nc.sync.dma_start(
                            t[:, c * d:(c + 1) * d], dr[c * P:(c + 1) * P, :])
                    ws[nm] = t

                def wview(nm, cin, cout):
                    return ws[nm][:, cin * d + cout * P: cin * d + (cout + 1) * P]

                # local X.T block (this core's rows): first R cols of the
                # rotated xt slab
                def xloc(t, c):
                    return kch(t, c)[:, 0:R]

                # KT = Wk~ @ X.T (3-term split), split into hi/lo
                for co in range(DC):
                    for jt in range(NJT):
                        ps = pa.tile([P, JT], F32, tag="pproj")
                        sl = slice(jt * JT, (jt + 1) * JT)
                        terms = [("wk_h", xth), ("wk_h", xtl), ("wk_l", xth)]
                        for ci in range(DC):
                            for ti, (wn, xs) in enumerate(terms):
                                nc.tensor.matmul(
                                    ps[:], wview(wn, ci, co),
                                    kch(xs, ci)[:, sl],
                                    start=(ci == 0 and ti == 0),
                                    stop=(ci == DC - 1 and ti == 2))
                        kh = kch(kt_h, co)[:, sl]
                        kl = kch(kt_l, co)[:, sl]
                        nc.scalar.copy(kh, ps[:])
                        nc.vector.tensor_sub(kl, ps[:], kh)

                # QT = Wq~ @ Xloc.T (3-term split)
                QJT = min(JT, R)
                for co in range(DC):
                    for it in range(R // QJT):
                        ps = pa.tile([P, QJT], F32, tag="pproj")
                        sl = slice(it * QJT, (it + 1) * QJT)
                        terms = [("wq_h", xth), ("wq_h", xtl), ("wq_l", xth)]
                        for ci in range(DC):
                            for ti, (wn, xs) in enumerate(terms):
                                nc.tensor.matmul(
                                    ps[:], wview(wn, ci, co),
                                    xloc(xs, ci)[:, sl],
                                    start=(ci == 0 and ti == 0),
                                    stop=(ci == DC - 1 and ti == 2))
                        qh = qch(qt_h, co)[:, sl]
                        ql = qch(qt_l, co)[:, sl]
                        nc.scalar.copy(qh, ps[:])
                        nc.vector.tensor_sub(ql, ps[:], qh)

                # global K stats via knat over HALF the rows (even j-blocks
                # only — the tau0 margins absorb the ~1% sampling noise).
                # K natural [j, d|1] bf16 with a ones column appended so the
                # C matmul also yields kbar (col sums) for free.
                NS = JB // 2  # sampled j-blocks
                DW = d + 1
                knat = xtp.tile([P, NS * DW], BF16, tag="xtl", name="knat")
                for bi, jb in enumerate(range(0, JB, 2)):
                    ps = pa.tile([P, d], F32, tag="pv")
                    for ci in range(DC):
                        nc.tensor.matmul(
                            ps[:], kch(xth, ci)[:, jb * P:(jb + 1) * P],
                            ws["wk_h"][:, ci * d:(ci + 1) * d],
                            start=(ci == 0), stop=(ci == DC - 1))
                    nc.scalar.copy(knat[:, bi * DW:bi * DW + d], ps[:])
                    if bi == 0:
                        # all NS ones columns in one strided memset
                        nc.vector.memset(
                            knat[:].rearrange(
                                "p (b w) -> p b w", w=DW)[:, :, d:DW], 1.0)
                # C[b-chunk cb][:, a] = sum_j K[j, b] K[j, a]; col d = kbar
                for cb in range(DC):
                    psC = pc.tile([P, DW], F32, tag="psC", name=f"psC_{cb}")
                    for bi in range(NS):
                        nc.tensor.matmul(
                            psC[:],
                            knat[:, bi * DW + cb * P: bi * DW + (cb + 1) * P],
                            knat[:, bi * DW:(bi + 1) * DW],
                            start=(bi == 0), stop=(bi == NS - 1))
                    nc.scalar.copy(c_sb[:, cb * d:(cb + 1) * d], psC[:, 0:d])
                    nc.scalar.copy(kbarb[:, cb:cb + 1], psC[:, d:DW])

                # V (+ones col), single bf16 term; all ones/pad columns
                # written by two strided memsets up front
                nc.vector.memset(
                    v_sb[:].rearrange(
                        "p (j w) -> p j w", w=VW)[:, :, d:d + 1], 1.0)
                nc.vector.memset(
                    v_sb[:].rearrange(
                        "p (j w) -> p j w", w=VW)[:, :, d + 1:VW], 0.0)
                for jb in range(JB):
                    ps = pa.tile([P, d], F32, tag="pv")
                    for ci in range(DC):
                        nc.tensor.matmul(
                            ps[:], kch(xth, ci)[:, jb * P:(jb + 1) * P],
                            ws["wv_h"][:, ci * d:(ci + 1) * d],
                            start=(ci == 0), stop=(ci == DC - 1))
                    base = jb * VW
                    nc.scalar.copy(v_sb[:, base:base + d], ps[:])

            # ---------------- stage B ----------------
            qpool = ctx.enter_context(tc.tile_pool(name="qpool", bufs=3))
            cpool = ctx.enter_context(tc.tile_pool(name="cpool", bufs=2))
            wpool = ctx.enter_context(tc.tile_pool(name="wpool", bufs=4))
            expool = ctx.enter_context(tc.tile_pool(name="expool", bufs=10))
            smal = ctx.enter_context(tc.tile_pool(name="smal", bufs=2))
            ps_s = ctx.enter_context(
                tc.tile_pool(name="ps_s", bufs=4, space="PSUM"))
            ps_o = ctx.enter_context(
                tc.tile_pool(name="ps_o", bufs=2, space="PSUM"))
            ps_m = ctx.enter_context(
                tc.tile_pool(name="ps_m", bufs=1, space="PSUM"))

            inv_n = 2.0 / n  # K moments are sampled over half the rows
            LVL = self.ablate
            SKIP = self.skip
            T = self.reps * NIB

            def phase1(t):
                ib = t % NIB
                isl = slice(ib * P, (ib + 1) * P)
                st = {"isl": isl, "ib": ib}

                # --- per-row stats: mu = q.kbar/n, var = q^T C q / n ---
                stat_ps = ps_m.tile([P, 2], F32, tag="stat_ps")
                mu_ps = stat_ps[:, 0:1]
                qs_ps = stat_ps[:, 1:2]
                pp = smal.tile([P, DC * P], BF16, tag="pp")
                for c in range(DC):
                    qb = qch(qt_h, c)[:, isl]
                    nc.tensor.matmul(
                        mu_ps[:], qb, kbarb[:, c:c + 1],
                        start=(c == 0), stop=(c == DC - 1))
                for ca in range(DC):
                    t1 = ps_m.tile([P, P], F32, tag="t1", name=f"t1_{t}_{ca}")
                    for cb in range(DC):
                        nc.tensor.matmul(
                            t1[:],
                            c_sb[:, cb * d + ca * P: cb * d + (ca + 1) * P],
                            qch(qt_h, cb)[:, isl],
                            start=(cb == 0), stop=(cb == DC - 1))
                    nc.vector.tensor_tensor(
                        pp[:, ca * P:(ca + 1) * P], t1[:],
                        qch(qt_h, ca)[:, isl], op=OP.mult)
                for ca in range(DC):
                    nc.tensor.matmul(
                        qs_ps[:], pp[:, ca * P:(ca + 1) * P], onebf[:],
                        start=(ca == 0), stop=(ca == DC - 1))
                mu = smal.tile([P, 1], F32, tag="mu")
                nc.scalar.activation(mu[:], mu_ps[:], ACTF.Copy, scale=inv_n)
                # sigma = sqrt(qs_ps) via bit-magic (+-3.5%, enough for the
                # tau0 margins); the exact sqrt(inv_n) = 1/64 factor is
                # folded into the z-scale constants below
                sigi = smal.tile([P, 1], I32, tag="sigi")
                nc.vector.tensor_scalar(
                    sigi[:], qs_ps[:].bitcast(I32), 1, None,
                    op0=OP.arith_shift_right)
                sig = smal.tile([P, 1], F32, tag="sig")
                nc.vector.tensor_scalar(
                    sig[:].bitcast(I32), sigi[:], 0x1fbd1df5, None,
                    op0=OP.add)
                lo_t = smal.tile([P, 1], F32, tag="lo_t")
                nc.vector.scalar_tensor_tensor(
                    lo_t[:], sig[:], self.z0 / 64.0, mu[:],
                    op0=OP.mult, op1=OP.add)
                nlo_t = smal.tile([P, 1], F32, tag="nlo_t")
                nc.vector.tensor_scalar_mul(nlo_t[:], lo_t[:], -1.0)
                # bisection runs in ex0 = f16(exp(s - lo_t)) space over
                # [1, hi_ex]; widths wrow[:, i] = (hi_ex - 1) * 2^-(i+1)
                t12 = smal.tile([P, 1], F32, tag="t12")
                nc.vector.tensor_scalar_mul(
                    t12[:], sig[:], (self.zhi - self.z0) / 64.0)
                hi_ex = smal.tile([P, 1], F32, tag="hi_ex")
                nc.scalar.activation(hi_ex[:], t12[:], ACTF.Exp)
                base = smal.tile([P, 1], F32, tag="base")
                nc.vector.tensor_scalar(
                    base[:], hi_ex[:], -1.0, None, op0=OP.add)
                wrow = smal.tile([P, ITERS], F32, tag="wrow")
                nc.vector.tensor_scalar(
                    wrow[:], conerow[:], base[:], None, op0=OP.mult)
                st["lo_t"], st["wrow"], st["nlo_t"] = lo_t, wrow, nlo_t

                if LVL < 1:
                    nc.sync.dma_start(out_d[isl, 0:1], lo_t[:])
                    return st

                # --- scores -> ex0 = f16(exp(s - lo_t)) straight from PSUM.
                # Selection and weights both run on ex0; the exp(lo_t - t)
                # rescale cancels in the softmax normalization.
                JQ = QW // JT  # jt tiles per quarter
                for qq in range(4):
                    exq = expool.tile([P, QW], F16, tag="exq")
                    st.setdefault("exq", []).append(exq)
                    for jt in range(JQ):
                        ps = ps_s.tile([P, JT], F32, tag="ps_s")
                        gsl = slice((qq * JQ + jt) * JT,
                                    (qq * JQ + jt + 1) * JT)
                        terms = [(qt_h, kt_h), (qt_h, kt_l), (qt_l, kt_h)]
                        if "terms" in SKIP:
                            terms = terms[:1]
                        NT = len(terms)
                        for c in range(DC):
                            for ti, (qs, ks) in enumerate(terms):
                                nc.tensor.matmul(
                                    ps[:], qch(qs, c)[:, isl],
                                    kch(ks, c)[:, gsl],
                                    start=(c == 0 and ti == 0),
                                    stop=(c == DC - 1 and ti == NT - 1))
                        nc.scalar.activation(
                            exq[:, jt * JT:(jt + 1) * JT], ps[:], ACTF.Exp,
                            bias=nlo_t[:], scale=1.0)
                    if qq == 0:
                        # self-exclusion: zero the ex0 diagonal (rotated:
                        # j == local row idx, always inside quarter 0)
                        doff = ib * P
                        nc.gpsimd.affine_select(
                            exq[:, doff:doff + P],
                            exq[:, doff:doff + P],
                            pattern=[[-1, P]], compare_op=OP.not_equal,
                            fill=0.0, base=0, channel_multiplier=1)

                if LVL < 2:
                    for qq in range(4):
                        nc.sync.dma_start(
                            out_d[isl, qq * 4:qq * 4 + 4],
                            st["exq"][qq][:, 0:8].bitcast(F32))
                    return st

                # --- compaction of candidates >= lo_t, per quarter ---
                cu16 = cpool.tile([P, W], U16, tag="cu16")
                st["cu16"] = cu16
                if "scatter" in SKIP:
                    nc.vector.memset(cu16[:], 0)
                for q in range(4):
                    exq = st["exq"][q]
                    idxp = qpool.tile([P, QW], I16, tag="idxp")
                    if "compact" in SKIP:
                        nc.vector.memset(idxp[:], -1)
                    else:
                        ind = qpool.tile([P, QW], F16, tag="indpos")
                        pos = qpool.tile([P, QW], F16, tag="indpos",
                                         name=f"pos_{t}_{q}")
                        nc.vector.tensor_scalar(
                            ind[:], exq[:], 1.0, None, op0=OP.is_ge)
                        nc.vector.tensor_tensor_scan(
                            pos[:], ind[:], ind[:], 0.0,
                            op0=OP.add, op1=OP.bypass)
                        # u = min(pos, CW-1) * ind (in place; fp16 exact
                        # <= 2048)
                        nc.vector.scalar_tensor_tensor(
                            pos[:], pos[:], float(CW - 1), ind[:],
                            op0=OP.min, op1=OP.mult)
                        # slot = u - 1; u=0 (non-candidate) -> -1, ignored
                        # by the scatter.
                        nc.vector.tensor_scalar(
                            idxp[:], pos[:], -1.0, None, op0=OP.add)
                    if LVL < 3:
                        nc.sync.dma_start(
                            out_d[isl, q:q + 1], idxp[:, 0:2].bitcast(F32))
                    if LVL >= 3 and "scatter" not in SKIP:
                        nc.gpsimd.local_scatter(
                            cu16[:, q * CW:(q + 1) * CW],
                            exq[:].bitcast(U16), idxp[:],
                            channels=P, num_elems=CW, num_idxs=QW)

                if LVL == 3:
                    cv = cu16[:].bitcast(F32).rearrange(
                        "p (q c) -> p q c", c=CW // 2)[:, :, 0:1]
                    dst = out_d[isl, 0:4].rearrange("p (q c) -> p q c", c=1)
                    nc.sync.dma_start(dst, cv)
                return st

            def phase23(t, st):
                if LVL < 4:
                    return
                isl = st["isl"]
                cu16 = st["cu16"]
                wrow = st["wrow"]

                # --- bisection for the k-th largest ex0 value ---
                # count' = sum sign(c - mid) over 4x96 slots = 2*cnt - 384
                # (zero slots count -1 since mid >= 1); cnt >= k iff
                # count' >= 2k - 384.
                C3 = cu16[:].bitcast(F16).rearrange(
                    "p (q w) -> p q w", w=CW)[:, :, 0:CV]
                scr = cpool.tile([P, 4 * CV], BF16, tag="bscr")
                scr3 = scr[:].rearrange("p (q w) -> p q w", w=CV)
                thr = float(2 * k - 4 * CV)
                lo = onef
                for it in range(0 if "bisect" in SKIP else ITERS):
                    w = wrow[:, it:it + 1]
                    nmid = smal.tile([P, 1], F32, tag="nmid")
                    nc.vector.scalar_tensor_tensor(
                        nmid[:], lo[:], -1.0, w, op0=OP.mult, op1=OP.subtract)
                    cnt = smal.tile([P, 1], F32, tag="cnt")
                    nc.scalar.activation(
                        scr3, C3, ACTF.Sign, bias=nmid[:], scale=1.0,
                        accum_out=cnt[:])
                    dcol = smal.tile([P, 1], F32, tag="dd")
                    nc.vector.scalar_tensor_tensor(
                        dcol[:], cnt[:], thr, w, op0=OP.is_ge, op1=OP.mult)
                    lo2 = smal.tile([P, 1], F32, tag="lo2")
                    nc.vector.tensor_tensor(lo2[:], lo[:], dcol[:], op=OP.add)
                    lo = lo2

                if LVL < 5:
                    nc.sync.dma_start(out_d[isl, 0:1], lo[:])
                    return

                # --- weights + transpose + AV ---
                po = ps_o.tile([P, VW], F32, tag="po")
                for q in range(4):
                    exq = st["exq"][q]
                    wq_t = wpool.tile([P, QW], BF16, tag="wtx")
                    if "weights" in SKIP:
                        nc.vector.memset(wq_t[:], 0.5)
                    else:
                        # wq = (ex0 >= lo) * ex0 -> bf16 weights; the
                        # per-row exp(lo_t) scale cancels in softmax
                        nc.vector.scalar_tensor_tensor(
                            wq_t[:], exq[:], lo[:], exq[:],
                            op0=OP.is_ge, op1=OP.mult)
                    if LVL < 6:
                        nc.sync.dma_start(
                            out_d[isl, q:q + 1], wq_t[:, 0:2].bitcast(F32))
                        continue
                    wt = wpool.tile([P, QW], BF16, tag="wtx",
                                    name=f"wt_{t}_{q}")
                    nc.sync.dma_start_transpose(
                        wt[:].rearrange("p (b f) -> p b f", f=P), wq_t[:])
                    if LVL < 7 or "av" in SKIP:
                        nc.sync.dma_start(
                            out_d[isl, q:q + 1], wt[:, 0:2].bitcast(F32))
                        continue
                    for jb in range(QW // P):
                        gjb = q * (QW // P) + jb
                        nc.tensor.matmul(
                            po[:], wt[:, jb * P:(jb + 1) * P],
                            v_sb[:, gjb * VW:(gjb + 1) * VW],
                            start=(gjb == 0), stop=(gjb == JB - 1))

                if LVL < 7 or "av" in SKIP:
                    return

                rec = smal.tile([P, 1], F32, tag="rec")
                nc.vector.reciprocal(rec[:], po[:, d:d + 1])
                o_sb = smal.tile([P, d], F32, tag="o_sb")
                nc.vector.tensor_scalar(
                    o_sb[:], po[:, 0:d], rec[:], None, op0=OP.mult)
                nc.sync.dma_start(out_d[isl, :], o_sb[:])

            states = {}
            for t in range(T + 1):
                if t >= 1:
                    phase23(t - 1, states[t - 1])
                if t < T:
                    states[t] = phase1(t)
                if t >= 1:
                    states.pop(t - 1)

        nc.compile()

    def make_in_maps(self, X, Wq, Wk, Wv, temperature):
        n, d, R = self.n, self.d, self.R
        r = 1.0 / (float(np.asarray(temperature).reshape(-1)[0]) * math.sqrt(d))
        XT = np.ascontiguousarray(np.asarray(X, np.float32).T)
        wq_h, wq_l = _bf16_split(np.ascontiguousarray(
            (np.asarray(Wq, np.float32) * r).T))
        wk_h, wk_l = _bf16_split(np.ascontiguousarray(
            np.asarray(Wk, np.float32).T))
        wv_h, _ = _bf16_split(np.ascontiguousarray(
            np.asarray(Wv, np.float32).T))
        in_maps = []
        for c in range(self.ncores):
            rot = np.roll(XT, -c * R, axis=1)
            xt_h, xt_l = _bf16_split(rot)
            in_maps.append({
                "xt_h": xt_h, "xt_l": xt_l,
                "wq_h": wq_h, "wq_l": wq_l, "wk_h": wk_h, "wk_l": wk_l,
                "wv_h": wv_h,
            })
        return in_maps

    def postprocess(self, results):
        return np.concatenate(
            [results[c]["out"] for c in range(self.ncores)], axis=0)


_programs = {}


def _get_program(n, d, k, ncores):
    key = (n, d, k, ncores)
    if key not in _programs:
        _programs[key] = Program(n, d, k, ncores)
    return _programs[key]


def kernel(example_features, Wq, bq, Wk, bk, Wv, bv, temperature, k):
    X = np.asarray(example_features, np.float32)
    n, d = X.shape
    k = int(k)
    for b in (bq, bk, bv):
        assert not np.any(np.asarray(b)), "nonzero biases unsupported"
    prog = _get_program(n, d, k, 8)
    in_maps = prog.make_in_maps(X, Wq, Wk, Wv, temperature)
    res = run_bass_kernel_spmd(prog.nc, in_maps, core_ids=list(range(8)))
    return prog.postprocess(res.results).astype(np.float32)


# revision 85
# speedup vs baseline: 1.4589x; 1.4589x over previous
"""Trainium2 Bass kernel for sparse (top-k) attention.

Reference computation (per row i of N=8192):
    Q = X @ Wq.T + bq ; K = X @ Wk.T + bk ; V = X @ Wv.T + bv
    S = (Q @ K.T) / (temp * sqrt(D)) ; S[i,i] = -1e9
    keep each row's top-k (k=64) scores, mask rest to -1e9, softmax, @ V.

Strategy (8 NeuronCores, SPMD, no collectives):
  - Rows of X sharded across cores (1024 each); K/V computed replicated
    on every core from the full X.
  - Per core the key axis (j) is rotated by c*1024 so the self-diagonal
    lands at compile-time-known columns (j == local row index).
  - Selection-critical matmuls (Q/K projections, Q@K.T) run as 3-term
    bf16 split products (hi*hi + hi*lo + lo*hi): ~1e-6 absolute score
    accuracy at 3 bf16 passes. The 1/(temp*sqrt(D)) scale is folded into
    Wq on the host.
  - Exact per-row k-th-largest threshold: per-row mean/std estimates
    (from q norms + global K moments) give a conservative candidate
    threshold tau0; candidates are stream-compacted (prefix scan +
    gpsimd local_scatter of fp32 values as u16 halves) into a 512-wide
    buffer; 16 rounds of count bisection (ACT Sign+accum counts, DVE
    updates) isolate the exact k-th value.
  - Weights w = exp(s - t) * [s >= t] (bf16), DMA-xbar transposed in
    128x128 blocks, then PE matmul against V augmented with a ones
    column -> numerator and denominator in one PSUM accumulation chain.
  - Stage B is emitted as a 2-phase software pipeline: P1(t) =
    stats/scores/compaction/scatter, P23(t-1) = bisect/weights/AV, so
    Pool scatters, DVE compaction, ACT counts and PE matmuls of
    adjacent row-blocks overlap. The score slab pool has 3 slots (1.5
    row-blocks) to decouple the phases.
"""

import sys

sys.path.insert(0, "/opt/trn_rl_repo/concourse")
sys.path.insert(0, "/opt/trn_rl_repo")

import math
from contextlib import ExitStack
from statistics import NormalDist

import ml_dtypes
import numpy as np

import concourse.bass as bass
import concourse.tile as tile
from concourse import bacc, mybir
from concourse.bass_utils import run_bass_kernel_spmd

F32 = mybir.dt.float32
BF16 = mybir.dt.bfloat16
I16 = mybir.dt.int16
I32 = mybir.dt.int32
F16 = mybir.dt.float16
U16 = mybir.dt.uint16
OP = mybir.AluOpType
ACTF = mybir.ActivationFunctionType
AX = mybir.AxisListType

NEG = -1e9
P = 128

ABLATE_LEVELS = (
    "stats", "scores", "compact_dve", "scatter", "bisect", "weights",
    "transpose", "full")


def _bf16_split(x):
    hi = np.asarray(x, np.float32).astype(ml_dtypes.bfloat16)
    lo = (x - hi.astype(np.float32)).astype(ml_dtypes.bfloat16)
    return np.ascontiguousarray(hi), np.ascontiguousarray(lo)


class Program:
    def __init__(self, n, d, k, ncores, iters=14, reps=1, ablate=None,
                 skip=()):
        assert n % (ncores * P) == 0 and d % P == 0
        self.n, self.d, self.k, self.ncores = n, d, k, ncores
        self.ablate = ABLATE_LEVELS.index(ablate or "full")
        self.skip = frozenset(skip)
        self.reps = reps
        self.R = n // ncores            # rows per core
        self.NIB = self.R // P          # i-blocks per core
        self.DC = d // P                # contraction chunks
        self.JT = 512                   # score matmul tile width
        self.NJT = n // self.JT
        self.JB = n // P                # 128-wide j blocks
        self.HW = n // 2                # slab half width
        self.QW = n // 4                # compaction quarter width
        self.CW = 128                   # compact slots per quarter (f32)
        self.W = 4 * self.CW            # total compact width
        self.CV = 96                    # counted slots per quarter
        self.VW = d + 2                 # V width incl. ones + pad col
        self.iters = iters
        tgt = min(2.25 * k, 0.45 * self.W)
        nd = NormalDist()
        self.z0 = nd.inv_cdf(1.0 - tgt / n)
        self.zhi = self.z0 + 1.2
        self.build()

    def build(self):
        n, d, k = self.n, self.d, self.k
        DC, NIB, JT, NJT, JB = self.DC, self.NIB, self.JT, self.NJT, self.JB
        HW, QW, CW, W, CV = self.HW, self.QW, self.CW, self.W, self.CV
        R, VW = self.R, self.VW
        ITERS = self.iters
        nc = self.nc = bacc.Bacc(
            "TRN2", target_bir_lowering=False, debug=False,
            num_devices=self.ncores)

        def din(name, shape, dt=BF16):
            return nc.dram_tensor(name, shape, dt, kind="ExternalInput").ap()

        xt_h = din("xt_h", (d, n))
        xt_l = din("xt_l", (d, n))
        wq_h = din("wq_h", (d, d))
        wq_l = din("wq_l", (d, d))
        wk_h = din("wk_h", (d, d))
        wk_l = din("wk_l", (d, d))
        wv_h = din("wv_h", (d, d))
        out_d = nc.dram_tensor("out", (R, d), F32, kind="ExternalOutput").ap()

        with tile.TileContext(nc) as tc, ExitStack() as ctx:
            # ---------------- persistent tensors ----------------
            pers = ctx.enter_context(tc.tile_pool(name="pers", bufs=1))
            kt_h = pers.tile([P, DC * n], BF16, tag="kt_h")
            kt_l = pers.tile([P, DC * n], BF16, tag="kt_l")
            v_sb = pers.tile([P, JB * VW], BF16, tag="v_sb")
            qt_h = pers.tile([P, DC * R], BF16, tag="qt_h")
            qt_l = pers.tile([P, DC * R], BF16, tag="qt_l")
            kbarb = pers.tile([P, DC], BF16, tag="kbarb")
            c_sb = pers.tile([P, DC * d], BF16, tag="c_sb")  # C = K^T K chunks
            conerow = pers.tile([P, ITERS], F32, tag="conerow")
            for it in range(ITERS):
                nc.vector.memset(conerow[:, it:it + 1], 0.5 ** (it + 1))
            onef = pers.tile([P, 1], F32, tag="onef")
            nc.vector.memset(onef[:], 1.0)
            onebf = pers.tile([P, 1], BF16, tag="onebf")
            nc.vector.memset(onebf[:], 1.0)

            def kch(t, c):
                return t[:, c * n:(c + 1) * n]

            def qch(t, c):
                return t[:, c * R:(c + 1) * R]

            # ---------------- stage A ----------------
            with tc.tile_pool(name="xt", bufs=1) as xtp, \
                 tc.tile_pool(name="wp", bufs=1) as wp, \
                 tc.tile_pool(name="pa", bufs=3, space="PSUM") as pa, \
                 tc.tile_pool(name="pc", bufs=1, space="PSUM") as pc:
                xth = xtp.tile([P, DC * n], BF16, tag="xth")
                xtl = xtp.tile([P, DC * n], BF16, tag="xtl")
                for c in range(DC):
                    nc.sync.dma_start(kch(xth, c), xt_h[c * P:(c + 1) * P, :])
                    nc.sync.dma_start(kch(xtl, c), xt_l[c * P:(c + 1) * P, :])
                ws = {}
                for nm, dr in [("wq_h", wq_h), ("wq_l", wq_l),
                               ("wk_h", wk_h), ("wk_l", wk_l),
                               ("wv_h", wv_h)]:
                    t = wp.tile([P, DC * d], BF16, tag=nm)
                    for c in range(DC):
                        nc.sync.dma_start(
                            t[:, c * d:(c + 1) * d], dr[c * P:(c + 1) * P, :])
                    ws[nm] = t

                def wview(nm, cin, cout):
                    return ws[nm][:, cin * d + cout * P: cin * d + (cout + 1) * P]

                # local X.T block (this core's rows): first R cols of the
                # rotated xt slab
                def xloc(t, c):
                    return kch(t, c)[:, 0:R]

                # KT = Wk~ @ X.T (3-term split), split into hi/lo
                for co in range(DC):
                    for jt in range(NJT):
                        ps = pa.tile([P, JT], F32, tag="pproj")
                        sl = slice(jt * JT, (jt + 1) * JT)
                        terms = [("wk_h", xth), ("wk_h", xtl), ("wk_l", xth)]
                        for ci in range(DC):
                            for ti, (wn, xs) in enumerate(terms):
                                nc.tensor.matmul(
                                    ps[:], wview(wn, ci, co),
                                    kch(xs, ci)[:, sl],
                                    start=(ci == 0 and ti == 0),
                                    stop=(ci == DC - 1 and ti == 2))
                        kh = kch(kt_h, co)[:, sl]
                        kl = kch(kt_l, co)[:, sl]
                        nc.scalar.copy(kh, ps[:])
                        nc.vector.tensor_sub(kl, ps[:], kh)

                # QT = Wq~ @ Xloc.T (3-term split)
                QJT = min(JT, R)
                for co in range(DC):
                    for it in range(R // QJT):
                        ps = pa.tile([P, QJT], F32, tag="pproj")
                        sl = slice(it * QJT, (it + 1) * QJT)
                        terms = [("wq_h", xth), ("wq_h", xtl), ("wq_l", xth)]
                        for ci in range(DC):
                            for ti, (wn, xs) in enumerate(terms):
                                nc.tensor.matmul(
                                    ps[:], wview(wn, ci, co),
                                    xloc(xs, ci)[:, sl],
                                    start=(ci == 0 and ti == 0),
                                    stop=(ci == DC - 1 and ti == 2))
                        qh = qch(qt_h, co)[:, sl]
                        ql = qch(qt_l, co)[:, sl]
                        nc.scalar.copy(qh, ps[:])
                        nc.vector.tensor_sub(ql, ps[:], qh)

                # global K stats via knat over HALF the rows (even j-blocks
                # only — the tau0 margins absorb the ~1% sampling noise).
                # K natural [j, d|1] bf16 with a ones column appended so the
                # C matmul also yields kbar (col sums) for free.
                NS = JB // 2  # sampled j-blocks
                DW = d + 1
                knat = xtp.tile([P, NS * DW], BF16, tag="xtl", name="knat")
                for bi, jb in enumerate(range(0, JB, 2)):
                    ps = pa.tile([P, d], F32, tag="pv")
                    for ci in range(DC):
                        nc.tensor.matmul(
                            ps[:], kch(xth, ci)[:, jb * P:(jb + 1) * P],
                            ws["wk_h"][:, ci * d:(ci + 1) * d],
                            start=(ci == 0), stop=(ci == DC - 1))
                    nc.scalar.copy(knat[:, bi * DW:bi * DW + d], ps[:])
                    if bi == 0:
                        # all NS ones columns in one strided memset
                        nc.vector.memset(
                            knat[:].rearrange(
                                "p (b w) -> p b w", w=DW)[:, :, d:DW], 1.0)
                # C[b-chunk cb][:, a] = sum_j K[j, b] K[j, a]; col d = kbar
                for cb in range(DC):
                    psC = pc.tile([P, DW], F32, tag="psC", name=f"psC_{cb}")
                    for bi in range(NS):
                        nc.tensor.matmul(
                            psC[:],
                            knat[:, bi * DW + cb * P: bi * DW + (cb + 1) * P],
                            knat[:, bi * DW:(bi + 1) * DW],
                            start=(bi == 0), stop=(bi == NS - 1))
                    nc.scalar.copy(c_sb[:, cb * d:(cb + 1) * d], psC[:, 0:d])
                    nc.scalar.copy(kbarb[:, cb:cb + 1], psC[:, d:DW])

                # V (+ones col), single bf16 term; all ones/pad columns
                # written by two strided memsets up front
                nc.vector.memset(
                    v_sb[:].rearrange(
                        "p (j w) -> p j w", w=VW)[:, :, d:d + 1], 1.0)
                nc.vector.memset(
                    v_sb[:].rearrange(
                        "p (j w) -> p j w", w=VW)[:, :, d + 1:VW], 0.0)
                for jb in range(JB):
                    ps = pa.tile([P, d], F32, tag="pv")
                    for ci in range(DC):
                        nc.tensor.matmul(
                            ps[:], kch(xth, ci)[:, jb * P:(jb + 1) * P],
                            ws["wv_h"][:, ci * d:(ci + 1) * d],
                            start=(ci == 0), stop=(ci == DC - 1))
                    base = jb * VW
                    nc.scalar.copy(v_sb[:, base:base + d], ps[:])

            # ---------------- stage B ----------------
            qpool = ctx.enter_context(tc.tile_pool(name="qpool", bufs=3))
            cpool = ctx.enter_context(tc.tile_pool(name="cpool", bufs=2))
            wpool = ctx.enter_context(tc.tile_pool(name="wpool", bufs=4))
            expool = ctx.enter_context(tc.tile_pool(name="expool", bufs=10))
            smal = ctx.enter_context(tc.tile_pool(name="smal", bufs=2))
            ps_s = ctx.enter_context(
                tc.tile_pool(name="ps_s", bufs=5, space="PSUM"))
            ps_o = ctx.enter_context(
                tc.tile_pool(name="ps_o", bufs=1, space="PSUM"))
            ps_m = ctx.enter_context(
                tc.tile_pool(name="ps_m", bufs=1, space="PSUM"))

            inv_n = 2.0 / n  # K moments are sampled over half the rows
            LVL = self.ablate
            SKIP = self.skip
            T = self.reps * NIB

            def phase1(t):
                ib = t % NIB
                isl = slice(ib * P, (ib + 1) * P)
                st = {"isl": isl, "ib": ib}

                # --- per-row stats: mu = q.kbar/n, var = q^T C q / n ---
                stat_ps = ps_m.tile([P, 2], F32, tag="stat_ps")
                mu_ps = stat_ps[:, 0:1]
                qs_ps = stat_ps[:, 1:2]
                pp = smal.tile([P, DC * P], BF16, tag="pp")
                for c in range(DC):
                    qb = qch(qt_h, c)[:, isl]
                    nc.tensor.matmul(
                        mu_ps[:], qb, kbarb[:, c:c + 1],
                        start=(c == 0), stop=(c == DC - 1))
                for ca in range(DC):
                    t1 = ps_m.tile([P, P], F32, tag="t1", name=f"t1_{t}_{ca}")
                    for cb in range(DC):
                        nc.tensor.matmul(
                            t1[:],
                            c_sb[:, cb * d + ca * P: cb * d + (ca + 1) * P],
                            qch(qt_h, cb)[:, isl],
                            start=(cb == 0), stop=(cb == DC - 1))
                    nc.vector.tensor_tensor(
                        pp[:, ca * P:(ca + 1) * P], t1[:],
                        qch(qt_h, ca)[:, isl], op=OP.mult)
                for ca in range(DC):
                    nc.tensor.matmul(
                        qs_ps[:], pp[:, ca * P:(ca + 1) * P], onebf[:],
                        start=(ca == 0), stop=(ca == DC - 1))
                mu = smal.tile([P, 1], F32, tag="mu")
                nc.scalar.activation(mu[:], mu_ps[:], ACTF.Copy, scale=inv_n)
                # sigma = sqrt(qs_ps) via bit-magic (+-3.5%, enough for the
                # tau0 margins); the exact sqrt(inv_n) = 1/64 factor is
                # folded into the z-scale constants below
                sigi = smal.tile([P, 1], I32, tag="sigi")
                nc.vector.tensor_scalar(
                    sigi[:], qs_ps[:].bitcast(I32), 1, None,
                    op0=OP.arith_shift_right)
                sig = smal.tile([P, 1], F32, tag="sig")
                nc.vector.tensor_scalar(
                    sig[:].bitcast(I32), sigi[:], 0x1fbd1df5, None,
                    op0=OP.add)
                lo_t = smal.tile([P, 1], F32, tag="lo_t")
                nc.vector.scalar_tensor_tensor(
                    lo_t[:], sig[:], self.z0 / 64.0, mu[:],
                    op0=OP.mult, op1=OP.add)
                nlo_t = smal.tile([P, 1], F32, tag="nlo_t")
                nc.vector.tensor_scalar_mul(nlo_t[:], lo_t[:], -1.0)
                # bisection runs in ex0 = f16(exp(s - lo_t)) space over
                # [1, hi_ex]; widths wrow[:, i] = (hi_ex - 1) * 2^-(i+1)
                t12 = smal.tile([P, 1], F32, tag="t12")
                nc.vector.tensor_scalar_mul(
                    t12[:], sig[:], (self.zhi - self.z0) / 64.0)
                hi_ex = smal.tile([P, 1], F32, tag="hi_ex")
                nc.scalar.activation(hi_ex[:], t12[:], ACTF.Exp)
                base = smal.tile([P, 1], F32, tag="base")
                nc.vector.tensor_scalar(
                    base[:], hi_ex[:], -1.0, None, op0=OP.add)
                wrow = smal.tile([P, ITERS], F32, tag="wrow")
                nc.vector.tensor_scalar(
                    wrow[:], conerow[:], base[:], None, op0=OP.mult)
                st["lo_t"], st["wrow"], st["nlo_t"] = lo_t, wrow, nlo_t

                if LVL < 1:
                    nc.sync.dma_start(out_d[isl, 0:1], lo_t[:])
                    return st

                # --- scores -> ex0 = f16(exp(s - lo_t)) straight from PSUM.
                # Selection and weights both run on ex0; the exp(lo_t - t)
                # rescale cancels in the softmax normalization.
                JQ = QW // JT  # jt tiles per quarter
                cu16 = cpool.tile([P, W], U16, tag="cu16")
                st["cu16"] = cu16
                if LVL >= 2 and "scatter" in SKIP:
                    nc.vector.memset(cu16[:], 0)
                # per quarter: scores -> exp -> compaction -> scatter fused
                # so Pool/DVE start on quarter q while PE runs quarter q+1
                for qq in range(4):
                    exq = expool.tile([P, QW], F16, tag="exq")
                    st.setdefault("exq", []).append(exq)
                    for jt in range(JQ):
                        ps = ps_s.tile([P, JT], F32, tag="ps_s")
                        gsl = slice((qq * JQ + jt) * JT,
                                    (qq * JQ + jt + 1) * JT)
                        terms = [(qt_h, kt_h), (qt_h, kt_l), (qt_l, kt_h)]
                        if "terms" in SKIP:
                            terms = terms[:1]
                        NT = len(terms)
                        for c in range(DC):
                            for ti, (qs, ks) in enumerate(terms):
                                nc.tensor.matmul(
                                    ps[:], qch(qs, c)[:, isl],
                                    kch(ks, c)[:, gsl],
                                    start=(c == 0 and ti == 0),
                                    stop=(c == DC - 1 and ti == NT - 1))
                        nc.scalar.activation(
                            exq[:, jt * JT:(jt + 1) * JT], ps[:], ACTF.Exp,
                            bias=nlo_t[:], scale=1.0)
                    if qq == 0:
                        # self-exclusion: zero the ex0 diagonal (rotated:
                        # j == local row idx, always inside quarter 0)
                        doff = ib * P
                        nc.gpsimd.affine_select(
                            exq[:, doff:doff + P],
                            exq[:, doff:doff + P],
                            pattern=[[-1, P]], compare_op=OP.not_equal,
                            fill=0.0, base=0, channel_multiplier=1)
                    if LVL < 2:
                        nc.sync.dma_start(
                            out_d[isl, qq * 4:qq * 4 + 4],
                            exq[:, 0:8].bitcast(F32))
                        continue
                    q = qq
                    idxp = qpool.tile([P, QW], I16, tag="idxp")
                    if "compact" in SKIP:
                        nc.vector.memset(idxp[:], -1)
                    else:
                        ind = qpool.tile([P, QW], F16, tag="indpos")
                        pos = qpool.tile([P, QW], F16, tag="indpos",
                                         name=f"pos_{t}_{q}")
                        nc.vector.tensor_scalar(
                            ind[:], exq[:], 1.0, None, op0=OP.is_ge)
                        nc.vector.tensor_tensor_scan(
                            pos[:], ind[:], ind[:], 0.0,
                            op0=OP.add, op1=OP.bypass)
                        # u = min(pos, CW-1) * ind (in place; fp16 exact
                        # <= 2048)
                        nc.vector.scalar_tensor_tensor(
                            pos[:], pos[:], float(CW - 1), ind[:],
                            op0=OP.min, op1=OP.mult)
                        # slot = u - 1; u=0 (non-candidate) -> -1, ignored
                        # by the scatter.
                        nc.vector.tensor_scalar(
                            idxp[:], pos[:], -1.0, None, op0=OP.add)
                    if LVL < 3:
                        nc.sync.dma_start(
                            out_d[isl, q:q + 1], idxp[:, 0:2].bitcast(F32))
                    if LVL >= 3 and "scatter" not in SKIP:
                        nc.gpsimd.local_scatter(
                            cu16[:, q * CW:(q + 1) * CW],
                            exq[:].bitcast(U16), idxp[:],
                            channels=P, num_elems=CW, num_idxs=QW)

                if LVL < 2:
                    return st
                if LVL == 3:
                    cv = cu16[:].bitcast(F32).rearrange(
                        "p (q c) -> p q c", c=CW // 2)[:, :, 0:1]
                    dst = out_d[isl, 0:4].rearrange("p (q c) -> p q c", c=1)
                    nc.sync.dma_start(dst, cv)
                return st

            def phase23(t, st):
                if LVL < 4:
                    return
                isl = st["isl"]
                cu16 = st["cu16"]
                wrow = st["wrow"]

                # --- bisection for the k-th largest ex0 value ---
                # count' = sum sign(c - mid) over 4x96 slots = 2*cnt - 384
                # (zero slots count -1 since mid >= 1); cnt >= k iff
                # count' >= 2k - 384.
                C3 = cu16[:].bitcast(F16).rearrange(
                    "p (q w) -> p q w", w=CW)[:, :, 0:CV]
                scr = cpool.tile([P, 4 * CV], BF16, tag="bscr")
                scr3 = scr[:].rearrange("p (q w) -> p q w", w=CV)
                thr = float(2 * k - 4 * CV)
                lo = onef
                for it in range(0 if "bisect" in SKIP else ITERS):
                    w = wrow[:, it:it + 1]
                    nmid = smal.tile([P, 1], F32, tag="nmid")
                    nc.vector.scalar_tensor_tensor(
                        nmid[:], lo[:], -1.0, w, op0=OP.mult, op1=OP.subtract)
                    cnt = smal.tile([P, 1], F32, tag="cnt")
                    nc.scalar.activation(
                        scr3, C3, ACTF.Sign, bias=nmid[:], scale=1.0,
                        accum_out=cnt[:])
                    dcol = smal.tile([P, 1], F32, tag="dd")
                    nc.vector.scalar_tensor_tensor(
                        dcol[:], cnt[:], thr, w, op0=OP.is_ge, op1=OP.mult)
                    lo2 = smal.tile([P, 1], F32, tag="lo2")
                    nc.vector.tensor_tensor(lo2[:], lo[:], dcol[:], op=OP.add)
                    lo = lo2

                if LVL < 5:
                    nc.sync.dma_start(out_d[isl, 0:1], lo[:])
                    return

                # --- weights + transpose + AV ---
                po = ps_o.tile([P, VW], F32, tag="po")
                for q in range(4):
                    exq = st["exq"][q]
                    wq_t = wpool.tile([P, QW], BF16, tag="wtx")
                    if "weights" in SKIP:
                        nc.vector.memset(wq_t[:], 0.5)
                    else:
                        # wq = (ex0 >= lo) * ex0 -> bf16 weights; the
                        # per-row exp(lo_t) scale cancels in softmax
                        nc.vector.scalar_tensor_tensor(
                            wq_t[:], exq[:], lo[:], exq[:],
                            op0=OP.is_ge, op1=OP.mult)
                    if LVL < 6:
                        nc.sync.dma_start(
                            out_d[isl, q:q + 1], wq_t[:, 0:2].bitcast(F32))
                        continue
                    wt = wpool.tile([P, QW], BF16, tag="wtx",
                                    name=f"wt_{t}_{q}")
                    nc.sync.dma_start_transpose(
                        wt[:].rearrange("p (b f) -> p b f", f=P), wq_t[:])
                    if LVL < 7 or "av" in SKIP:
                        nc.sync.dma_start(
                            out_d[isl, q:q + 1], wt[:, 0:2].bitcast(F32))
                        continue
                    for jb in range(QW // P):
                        gjb = q * (QW // P) + jb
                        nc.tensor.matmul(
                            po[:], wt[:, jb * P:(jb + 1) * P],
                            v_sb[:, gjb * VW:(gjb + 1) * VW],
                            start=(gjb == 0), stop=(gjb == JB - 1))

                if LVL < 7 or "av" in SKIP:
                    return

                rec = smal.tile([P, 1], F32, tag="rec")
                nc.vector.reciprocal(rec[:], po[:, d:d + 1])
                o_sb = smal.tile([P, d], F32, tag="o_sb")
                nc.vector.tensor_scalar(
                    o_sb[:], po[:, 0:d], rec[:], None, op0=OP.mult)
                nc.sync.dma_start(out_d[isl, :], o_sb[:])

            states = {}
            for t in range(T + 1):
                if t >= 1:
                    phase23(t - 1, states[t - 1])
                if t < T:
                    states[t] = phase1(t)
                if t >= 1:
                    states.pop(t - 1)

        nc.compile()

    def make_in_maps(self, X, Wq, Wk, Wv, temperature):
        n, d, R = self.n, self.d, self.R
        r = 1.0 / (float(np.asarray(temperature).reshape(-1)[0]) * math.sqrt(d))
        XT = np.ascontiguousarray(np.asarray(X, np.float32).T)
        wq_h, wq_l = _bf16_split(np.ascontiguousarray(
            (np.asarray(Wq, np.float32) * r).T))
        wk_h, wk_l = _bf16_split(np.ascontiguousarray(
            np.asarray(Wk, np.float32).T))
        wv_h, _ = _bf16_split(np.ascontiguousarray(
            np.asarray(Wv, np.float32).T))
        in_maps = []
        for c in range(self.ncores):
            rot = np.roll(XT, -c * R, axis=1)
            xt_h, xt_l = _bf16_split(rot)
            in_maps.append({
                "xt_h": xt_h, "xt_l": xt_l,
                "wq_h": wq_h, "wq_l": wq_l, "wk_h": wk_h, "wk_l": wk_l,
                "wv_h": wv_h,
            })
        return in_maps

    def postprocess(self, results):
        return np.concatenate(
            [results[c]["out"] for c in range(self.ncores)], axis=0)


_programs = {}


def _get_program(n, d, k, ncores):
    key = (n, d, k, ncores)
    if key not in _programs:
        _programs[key] = Program(n, d, k, ncores)
    return _programs[key]


def kernel(example_features, Wq, bq, Wk, bk, Wv, bv, temperature, k):
    X = np.asarray(example_features, np.float32)
    n, d = X.shape
    k = int(k)
    for b in (bq, bk, bv):
        assert not np.any(np.asarray(b)), "nonzero biases unsupported"
    prog = _get_program(n, d, k, 8)
    in_maps = prog.make_in_maps(X, Wq, Wk, Wv, temperature)
    res = run_bass_kernel_spmd(prog.nc, in_maps, core_ids=list(range(8)))
    return prog.postprocess(res.results).astype(np.float32)
